# revision 1
# baseline (speedup 1.0000x reference)
"""Trainium2 Bass kernel for CategorySpecificLinear (MoE-style routed linear).

out[i] = x[i] @ W[cat_ids[i]] + b[cat_ids[i]]
  x: [64, 256, 1024] f32, cat_ids: [64] int, W: [16, 1024, 4096] f32,
  b: [16, 4096] f32  ->  out: [64, 256, 4096] f32

Sharding: data-parallel over batch. Rows are sorted by cat_id and chunked
8 per core; weights are gathered host-side per row; x is transposed
host-side so the contraction dim lands on SBUF partitions. Matmuls run in
float32r (full-rate on the PE at N=512, ~1e-4 relative accuracy). The
bias is added host-side after the gather.
"""

import numpy as np

import concourse.bass as bass
import concourse.mybir as mybir

F32 = mybir.dt.float32
F32R = mybir.dt.float32r

NCORES = 8
NROW = 8          # batch rows per core
SEQ = 256
KDIM = 1024       # input dim
NDIM = 4096       # hidden dim
KT = KDIM // 128  # k-tiles
NT = NDIM // 512  # n-slices
MT = SEQ // 128   # m-tiles per row


def _fix_multi_waits(nc, max_waits=1):
    """The walrus build here rejects instructions carrying more than one
    sync-wait command; split extra waits onto single-wait NOPs inserted
    before the instruction on the same engine (same-engine waits execute
    in order, so this is semantics-preserving)."""
    for f in nc.m.functions:
        for blk in f.blocks:
            il = blk.instructions
            i = 0
            while i < len(il):
                inst = il[i]
                si = getattr(inst, "sync_info", None)
                if si is not None and len(si.on_wait) > max_waits:
                    waits = list(si.on_wait)
                    keep, extra = waits[-max_waits:], waits[:-max_waits]
                    for w in extra:
                        nop = mybir.InstNoOp(
                            name=nc.get_next_instruction_name(),
                            sync_info=mybir.SyncInfo(on_wait=[w], on_update=[]),
                            bass_nofuse=True,
                            engine=inst.engine,
                        )
                        nc.register_instruction(nop, overwrite=True)
                        il.insert(i, nop)
                        i += 1
                    inst.sync_info = mybir.SyncInfo(
                        on_wait=keep, on_update=list(si.on_update)
                    )
                i += 1


def _build_program():
    from concourse import tile

    nc = bass.Bass()
    xt_d = nc.declare_dram_parameter("xt", [KDIM, NROW * SEQ], F32R, isOutput=False)
    w_d = nc.declare_dram_parameter("w", [NROW * KDIM, NDIM], F32R, isOutput=False)
    out_d = nc.declare_dram_parameter("out", [NROW * SEQ, NDIM], F32, isOutput=True)

    with tile.TileContext(nc) as tc:
        with (
            tc.tile_pool(name="xt", bufs=1) as xt_pool,
            tc.tile_pool(name="wbuf", bufs=3) as w_pool,
            tc.tile_pool(name="ostage", bufs=4) as o_pool,
            tc.tile_pool(name="psum", bufs=4, space="PSUM") as p_pool,
        ):
            xt_sb = []
            for kk in range(KT):
                t = xt_pool.tile([128, NROW * SEQ], F32R, tag=f"xt{kk}", name=f"xt{kk}")
                nc.sync.dma_start(out=t[:], in_=xt_d[kk * 128:(kk + 1) * 128, :])
                xt_sb.append(t)

            for r in range(NROW):
                ostages = []
                for m in range(MT):
                    ot = o_pool.tile([128, NDIM], F32, tag="ostage", name=f"os{r}_{m}")
                    ostages.append(ot)
                for n in range(NT):
                    wb = w_pool.tile(
                        [128, KT * 512], F32R, tag="wbuf", name=f"wb{r}_{n}"
                    )
                    src = w_d[r * KDIM:(r + 1) * KDIM, n * 512:(n + 1) * 512]
                    src = src.rearrange("(kk p) f -> p kk f", p=128)
                    dst = wb[:].rearrange("p (kk f) -> p kk f", kk=KT)
                    nc.sync.dma_start(out=dst, in_=src)
                    for m in range(MT):
                        ps = p_pool.tile([128, 512], F32, tag="psum", name=f"ps{r}{n}{m}")
                        moff = r * SEQ + m * 128
                        for kk in range(KT):
                            nc.tensor.matmul(
                                ps[:],
                                xt_sb[kk][:, moff:moff + 128],
                                wb[:, kk * 512:(kk + 1) * 512],
                                start=(kk == 0),
                                stop=(kk == KT - 1),
                            )
                        nc.vector.tensor_copy(
                            ostages[m][:, n * 512:(n + 1) * 512], ps[:]
                        )
                for m in range(MT):
                    moff = r * SEQ + m * 128
                    nc.scalar.dma_start(
                        out=out_d[moff:moff + 128, :], in_=ostages[m][:]
                    )
    _fix_multi_waits(nc)
    return nc


_PROGRAM_CACHE = {}


def _get_program():
    if "nc" not in _PROGRAM_CACHE:
        _PROGRAM_CACHE["nc"] = _build_program()
    return _PROGRAM_CACHE["nc"]


def kernel(x, cat_ids, W, b):
    from concourse.bass_utils import run_bass_kernel_spmd

    x = np.asarray(x)
    cat_ids_np = np.asarray(cat_ids).astype(np.int64)
    W = np.asarray(W)
    b = np.asarray(b)
    B = x.shape[0]
    assert x.shape == (B, SEQ, KDIM) and B == NCORES * NROW

    perm = np.argsort(cat_ids_np, kind="stable")
    in_maps = []
    for c in range(NCORES):
        rows = perm[c * NROW:(c + 1) * NROW]
        xt = np.ascontiguousarray(
            x[rows].transpose(2, 0, 1).reshape(KDIM, NROW * SEQ)
        )
        ws = np.ascontiguousarray(
            W[cat_ids_np[rows]].reshape(NROW * KDIM, NDIM)
        )
        in_maps.append({"xt": xt, "w": ws})

    nc = _get_program()
    res = run_bass_kernel_spmd(nc, in_maps, list(range(NCORES)), trace=False)

    out = np.empty((B, SEQ, NDIM), dtype=np.float32)
    for c in range(NCORES):
        rows = perm[c * NROW:(c + 1) * NROW]
        out[rows] = res.results[c]["out"].reshape(NROW, SEQ, NDIM)
    out += b[cat_ids_np][:, None, :]
    return out
